# revision 46
# baseline (speedup 1.0000x reference)
"""Trainium2 Bass kernel: gated MoE residual block (two 3x3 convs, C=32).

  g  = gate * (gate > 0)                          # [B, C]
  h  = relu((conv3x3(x, w1) + b1) * g)
  h2 = relu((conv3x3(h, w2) + b2) * g)
  out = h2 + x

Sharding: data-parallel over batch. 16 images -> 8 cores x 2 images.

Device algorithm (per core, per image), fp8 e4m3 + DoubleRow matmuls:
  - x arrives host-packed in "mod-4 row-interleaved" fp8 SBUF layout:
    partition 32*(row%4)+ci, free = (slot=row//4+1, col+1), zero halo
    baked in (slots 0, 65, 66 and cols 0, 257 are zero).
  - conv as DoubleRow fp8 matmul PAIRS (cost-model rate 0.5 cy/row,
    two K=128 matmuls fused): per 4-row window, 3 pairs with N=256:
      P0 = (main dx0, main dx1)   rhs k-tiles 1 col apart
      P1 = (main dx2, wrap dx0)   rhs k-tiles WP-2 apart (next slot)
      P2 = (wrap dx1, wrap dx2)   rhs k-tiles 1 col apart
    The paired rhs view is one 3-dim AP [128, 2, 256] built by giving
    the k-tile dim a custom stride over the same x_il tile, so no
    second x copy (and no x_wrap HBM read) is needed.
  - weights pre-scaled by 64 into fp8 range; epilogue scale = g/64.
  - conv1 epilogue on ScalarE: h = relu(psum*g/64 + b1*g) -> fp8 h_il.
  - conv2 epilogue on VectorE in shifted max-form (1 instruction):
    h2' = max(psum*g/64, -b2*g) = relu(psum*g/64 + b2*g) - b2*g.
    The constant b2*g is restored host-side together with the residual:
    out = h2' + b2*g + x  (all f32 on host). The two tail chunks run on
    ScalarE in true-relu form instead (bias pre-applied; the host skips
    the +b2*g for their rows) so the drain never serializes on VectorE.
  - PSUM: one 4-buffer pool of [128, 4, 256] tiles (4 windows each)
    shared by both convs; conv1/conv2 pipelined with a 2-chunk lag and
    the two images software-pipelined so conv2's tail drains under the
    next image's conv1; epilogues batch 4 windows/instr.
  - out staged fp8, stored in 8-slot chunks (Pool engine) with finer
    tail stores spread over SP so no engine queue delays the drain.
"""

import numpy as np
import ml_dtypes

import concourse.bass as bass
import concourse.tile as tile
from concourse import bacc, mybir

B, C, H, W = 16, 32, 256, 256
IMGS_PER_CORE = 2
N_CORES = 8
KW = 3
S = 4            # row interleave factor (rows per window/slot)
A = H // S       # 64 aligned 4-row windows
WP = W + 2       # padded row width (zero cols 0 and 257)
NSX = A + 3      # x_il slots: idx = window + 1; idx 0, A+1, A+2 zero
NSR = A + 2      # out_stage slots
F32 = mybir.dt.float32
FP8 = mybir.dt.float8e4
E4 = ml_dtypes.float8_e4m3
WSCALE = 64.0    # weight pre-scale into fp8-normal range
DR = mybir.MatmulPerfMode.DoubleRow

# conv windows k = -1 .. 63, in chunks of (up to) 4; the tail is split
# finer so the post-conv1 drain (conv2 mm -> ep -> store) pipelines short
CHUNKS = ([[-1, 0, 1, 2]]
          + [list(range(s, s + 4)) for s in range(3, 56, 4)]
          + [[59, 60], [61, 62], [A - 1]])
# x_il slot ranges per DMA chunk; conv chunk c needs slots <= 4c+4, and
# boundaries are tuned so each chunk lands just before the PE consumes it
XCHUNKS = [(0, 3), (3, 7), (7, 15), (15, 25), (25, 41), (41, 57), (57, NSX)]


def _pack_weights(w: np.ndarray) -> np.ndarray:
    """w: [C_out, C_in, 3, 3] (OIHW) -> [6, 128, 128] lhsT stack.

    Block (s, q) of main[dx] = w[:, :, s-q, dx].T   (0 <= s-q <= 2)
    Block (s, q) of wrap[dx] = w[:, :, 4+s-q, dx].T (0 <= 4+s-q <= 2)
    lhsT[(32s+ci), (32q+co)]; out row (window k) = 4k+1+q.
    """
    wv = np.zeros((2 * KW, S * C, S * C), dtype=np.float32)
    for dx in range(KW):
        for q in range(S):
            for s in range(S):
                if 0 <= s - q <= 2:
                    wv[2 * dx, 32 * s:32 * s + 32, 32 * q:32 * q + 32] = \
                        w[:, :, s - q, dx].T
                if 0 <= 4 + s - q <= 2:
                    wv[2 * dx + 1, 32 * s:32 * s + 32, 32 * q:32 * q + 32] = \
                        w[:, :, 4 + s - q, dx].T
    return wv


# DoubleRow pair -> (full[] index of k-tile0, k-tile1)
PAIR_IDX = [(0, 2), (4, 1), (3, 5)]   # (m0,m1), (m2,w0), (w1,w2)


def _pack_pairs(w: np.ndarray) -> np.ndarray:
    """-> [128, 3, 2, 128] fp8 lhsT pair stack (K partition-first)."""
    full = _pack_weights(w) * WSCALE
    wvp = np.stack([np.stack([full[a], full[b]]) for a, b in PAIR_IDX])
    # [3, 2, K, M] -> [K, 3, 2, M]
    return np.ascontiguousarray(wvp.transpose(2, 0, 1, 3)).astype(E4)


def _interleave_x(x: np.ndarray) -> np.ndarray:
    """x: [n, C, H, W] f32 -> x_il [n, 128, NSX, WP] fp8.

    x_il: partition 32s+ci holds row 4(i-1)+s at slot i, col c+1.
    """
    n = x.shape[0]
    xq = x.astype(E4)
    ext = np.zeros((n, C, S * NSX, W), dtype=E4)
    ext[:, :, S:S + H, :] = xq
    il = ext.reshape(n, C, NSX, S, W).transpose(0, 3, 1, 2, 4) \
            .reshape(n, S * C, NSX, W)
    x_il = np.zeros((n, S * C, NSX, WP), dtype=E4)
    x_il[:, :, :, 1:1 + W] = il
    return np.ascontiguousarray(x_il)


def _deinterleave_out(dev: np.ndarray) -> np.ndarray:
    """dev: [n, 128, NSR, W] (row z = 4(i-1)+2+q at partition 32q+co)
    -> [n, C, H, W] f32."""
    dev = np.asarray(dev).astype(np.float32)
    n = dev.shape[0]
    v = dev.reshape(n, S, C, NSR, W).transpose(0, 2, 3, 1, 4) \
           .reshape(n, C, S * NSR, W)
    return np.ascontiguousarray(v[:, :, 2:2 + H, :])


def _build_core_graph():
    nc = bacc.Bacc(None, target_bir_lowering=False, debug=False)

    xil_ext = nc.declare_dram_parameter(
        "xil", [IMGS_PER_CORE, S * C, NSX, WP], FP8, isOutput=False)
    wv1_ext = nc.declare_dram_parameter(
        "wv1", [S * C, KW, 2, S * C], FP8, isOutput=False)
    wv2_ext = nc.declare_dram_parameter(
        "wv2", [S * C, KW, 2, S * C], FP8, isOutput=False)
    # sc cols: 0-1 g/WSCALE, 2-3 b1*g, 4-5 -(b2*g)   (per image)
    sc_ext = nc.declare_dram_parameter("sc", [S * C, 8], F32, isOutput=False)
    out_ext = nc.declare_dram_parameter(
        "out", [IMGS_PER_CORE, S * C, NSR, W], FP8, isOutput=True)

    RELU = mybir.ActivationFunctionType.Relu
    MULT = mybir.AluOpType.mult
    MAX = mybir.AluOpType.max

    with tile.TileContext(nc) as tc:
        with (
            tc.tile_pool(name="const", bufs=1) as cpool,
            tc.tile_pool(name="xb", bufs=2) as xpool,
            tc.tile_pool(name="hb", bufs=1) as hpool,
            tc.tile_pool(name="os", bufs=2) as ospool,
            tc.tile_pool(name="pp", bufs=4, space=bass.MemorySpace.PSUM) as psp,
        ):
            wv1_t = cpool.tile([S * C, KW, 2, S * C], FP8)
            wv2_t = cpool.tile([S * C, KW, 2, S * C], FP8)
            sc_t = cpool.tile([S * C, 8], F32)
            h_il = hpool.tile([S * C, NSX, WP], FP8)

            # consts spread over Pool/ACT so SP's serial queue is pure x
            # streaming (first matmul needs wv1 + x slots 0:3 ASAP); wv2 on
            # ACT behind its act-table load, still ready before conv2
            nc.gpsimd.dma_start(out=wv1_t[:], in_=wv1_ext[:])
            nc.gpsimd.dma_start(out=sc_t[:], in_=sc_ext[:])
            nc.scalar.dma_start(out=wv2_t[:], in_=wv2_ext[:])

            # h halo: never written by epilogues, init once (Pool engine).
            # slot 0 (window -1 writes only partitions 96:128 later), the
            # q=3 strip of slot 64 (row 256 pad), slots 65/66, cols 0/257.
            nc.gpsimd.memset(h_il[:, 0, :], 0.0)
            nc.gpsimd.memset(h_il[3 * C:4 * C, A, :], 0.0)
            nc.gpsimd.memset(h_il[:, A + 1, :], 0.0)
            nc.gpsimd.memset(h_il[:, A + 2, :], 0.0)
            nc.gpsimd.memset(h_il[:, :, 0], 0.0)
            nc.gpsimd.memset(h_il[:, :, WP - 1], 0.0)

            def pair_rhs(src, sl, col, delta):
                base = src[:, sl, col:col + W]
                v = base.unsqueeze(1).broadcast_to((S * C, 2, W))
                av = v.ap
                av[1] = [delta, 2]
                v.ap = av
                return v

            # pair p -> (base slot offset from k, base col, k-tile delta)
            PAIR_GEO = [(1, 0, 1), (1, 2, WP - 2), (2, 1, 1)]

            def conv_chunk(src, wv_t, ws, ps, conv_idx):
                for j, k in enumerate(ws):
                    if k == -1:
                        plist = [1, 2] if conv_idx == 1 else [0, 1, 2]
                    elif k == A - 1:
                        plist = [0, 1]
                    else:
                        plist = [0, 1, 2]
                    for n, p in enumerate(plist):
                        soff, col, delta = PAIR_GEO[p]
                        nc.tensor.matmul(
                            ps[:, j, :], wv_t[:, p, :, :],
                            pair_rhs(src, k + soff, col, delta),
                            start=(n == 0), stop=(n == len(plist) - 1),
                            perf_mode=DR, skip_group_check=True)

            def ep1(ps, ws, img):
                # h = relu(psum*g/64 + b1*g), true form, fp8 into h_il.
                # One full-range instruction per chunk (split partition
                # ranges invite scheduler reordering that stalls the PE);
                # the halo strip clobbered by window -1 is re-zeroed after.
                # The single-window tail chunk writes partitions 0:96 only
                # so the slot-64 halo strip stays pristine (a re-zero there
                # would gate the last conv2 matmuls)
                s0 = ws[0] + 1
                p1 = 3 * C if ws[-1] == A - 1 else 4 * C
                nc.scalar.activation(
                    h_il[0:p1, s0:s0 + len(ws), 1:1 + W],
                    ps[0:p1, 0:len(ws), :], RELU,
                    bias=sc_t[0:p1, 2 + img:3 + img],
                    scale=sc_t[0:p1, img:img + 1])
                if ws[0] == -1:
                    nc.gpsimd.memset(h_il[0:3 * C, 0, :], 0.0)

            def ep2(ps, ws, img, out_stage, eng):
                # h2' = max(psum*g/64, -b2*g); host adds back b2*g (+x).
                # eng=scalar: true-relu form on ScalarE (tail only); host
                # skips +b2*g for its rows (254-255)
                # full 128-partition writes: the discarded edge rows get
                # garbage, which the host slices off anyway
                s0 = ws[0] + 1
                nj = len(ws)
                if eng is nc.scalar:
                    nc.scalar.activation(
                        out_stage[:, s0:s0 + nj, :],
                        ps[:, 0:nj, :], RELU,
                        bias=sc_t[:, 6 + img:7 + img],
                        scale=sc_t[:, img:img + 1])
                    return
                negb = sc_t[:, 4 + img:5 + img] \
                    .unsqueeze(2).broadcast_to((S * C, nj, W))
                eng.scalar_tensor_tensor(
                    out_stage[:, s0:s0 + nj, :],
                    ps[:, 0:nj, :],
                    sc_t[:, img:img + 1],
                    negb, MULT, MAX)

            NCH = len(CHUNKS)
            stage = {}  # img -> (x_il, out_stage, last_stored_slot)

            def setup_img(img):
                x_il = xpool.tile([S * C, NSX, WP], FP8)
                out_stage = ospool.tile([S * C, NSR, W], FP8)
                for c0, c1 in XCHUNKS:
                    nc.sync.dma_start(out=x_il[:, c0:c1, :],
                                      in_=xil_ext[img, :, c0:c1, :])
                stage[img] = [x_il, out_stage, 0]

            def conv1_chunk(img, ci):
                ws = CHUNKS[ci]
                ps = psp.tile([S * C, S, W], F32, tag="ps")
                conv_chunk(stage[img][0], wv1_t, ws, ps, 1)
                ep1(ps, ws, img)

            def conv2_chunk(img, ci):
                ws = CHUNKS[ci]
                out_stage = stage[img][1]
                ps = psp.tile([S * C, S, W], F32, tag="ps")
                conv_chunk(h_il, wv2_t, ws, ps, 2)
                # GPSIMD cannot touch PSUM (BIR verifier), so epilogues live
                # on DVE with the c15/c17 tail chunks on ScalarE (true-relu
                # form) so the drain doesn't serialize through DVE
                if ci == NCH - 1 or ci == NCH - 3:
                    eng = nc.scalar
                else:
                    eng = nc.vector
                ep2(ps, ws, img, out_stage, eng)
                # store completed slot ranges (out slot = window+1): 8-slot
                # chunks mid-stream from Pool; the tail stores spread over
                # SP/Pool/ACT so no engine queue delays the drain
                if ci == NCH - 1:
                    nc.sync.dma_start(
                        out=out_ext[img, :, A:A + 1, :],
                        in_=out_stage[:, A:A + 1, :])
                    return
                hi = ws[-1] + 2
                lo = stage[img][2]
                if ci == NCH - 5 or ci == NCH - 2:
                    st_eng = nc.sync
                elif ci == NCH - 4 or hi - lo >= 8:
                    st_eng = nc.gpsimd
                else:
                    st_eng = None
                if st_eng is not None:
                    st_eng.dma_start(
                        out=out_ext[img, :, lo:hi, :],
                        in_=out_stage[:, lo:hi, :])
                    stage[img][2] = hi

            # flat software pipeline over both images: conv2 lags conv1 by
            # 2 chunks (its h slots come from conv1 chunk ci+1's epilogue),
            # and each image's conv2 tail drains under the next image's
            # conv1 head so the PE never idles at the boundary
            setup_img(0)
            for img in range(IMGS_PER_CORE):
                if img + 1 < IMGS_PER_CORE:
                    setup_img(img + 1)
                for ci in range(NCH):
                    conv1_chunk(img, ci)
                    if ci >= 2:
                        conv2_chunk(img, ci - 2)
                    elif img > 0:
                        conv2_chunk(img - 1, NCH - 2 + ci)
            conv2_chunk(IMGS_PER_CORE - 1, NCH - 2)
            conv2_chunk(IMGS_PER_CORE - 1, NCH - 1)

    nc.compile()
    return nc


def _host_prep(x, gate_values, w1, b1, w2, b2):
    x = np.ascontiguousarray(np.asarray(x, dtype=np.float32))
    gate_values = np.asarray(gate_values, dtype=np.float32)
    w1 = np.asarray(w1, dtype=np.float32)
    b1 = np.asarray(b1, dtype=np.float32)
    w2 = np.asarray(w2, dtype=np.float32)
    b2 = np.asarray(b2, dtype=np.float32)

    g = gate_values * (gate_values > 0)                      # [B, C]
    wv1 = _pack_pairs(w1)
    wv2 = _pack_pairs(w2)

    in_maps = []
    for core in range(N_CORES):
        sl = slice(core * IMGS_PER_CORE, (core + 1) * IMGS_PER_CORE)
        gc = g[sl]                                           # [2, C]
        sc = np.zeros((S * C, 8), dtype=np.float32)
        sc[:, 0:2] = np.tile((gc / WSCALE).T, (S, 1))
        sc[:, 2:4] = np.tile((gc * b1[None, :]).T, (S, 1))
        sc[:, 4:6] = np.tile((-gc * b2[None, :]).T, (S, 1))
        sc[:, 6:8] = np.tile((gc * b2[None, :]).T, (S, 1))
        in_maps.append({
            "xil": _interleave_x(x[sl]),
            "wv1": wv1, "wv2": wv2,
            "sc": np.ascontiguousarray(sc),
        })
    return in_maps


_NC_CACHE = None


def _get_graph():
    global _NC_CACHE
    if _NC_CACHE is None:
        _NC_CACHE = _build_core_graph()
    return _NC_CACHE


def kernel(x, gate_values, w1, b1, w2, b2, _trace=False, **_ignored):
    from concourse.bass_utils import run_bass_kernel_spmd

    nc = _get_graph()
    in_maps = _host_prep(x, gate_values, w1, b1, w2, b2)
    res = run_bass_kernel_spmd(
        nc, in_maps, core_ids=list(range(N_CORES)), trace=_trace)
    outs = [_deinterleave_out(res.results[i]["out"]) for i in range(N_CORES)]
    full = np.concatenate(outs, axis=0).astype(np.float32)
    # restore the shifted conv2 bias and add the residual (f32, host-side).
    # rows whose tail epilogue ran on ScalarE in true-relu form already
    # have the bias applied, so they are excluded here
    g = np.asarray(gate_values, dtype=np.float32)
    g = g * (g > 0)
    bg2 = (g * np.asarray(b2, dtype=np.float32)[None, :])[:, :, None, None]
    mask = np.ones(H, dtype=bool)
    for w in CHUNKS[-3] + CHUNKS[-1]:
        mask[4 * w + 2:4 * w + 6] = False
    full[:, :, mask, :] += bg2
    full += np.asarray(x, dtype=np.float32)
    if _trace:
        return full, res
    return full


# revision 49
# speedup vs baseline: 1.0041x; 1.0041x over previous
"""Trainium2 Bass kernel: gated MoE residual block (two 3x3 convs, C=32).

  g  = gate * (gate > 0)                          # [B, C]
  h  = relu((conv3x3(x, w1) + b1) * g)
  h2 = relu((conv3x3(h, w2) + b2) * g)
  out = h2 + x

Sharding: data-parallel over batch. 16 images -> 8 cores x 2 images.

Device algorithm (per core, per image), fp8 e4m3 + DoubleRow matmuls:
  - x arrives host-packed in "mod-4 row-interleaved" fp8 SBUF layout:
    partition 32*(row%4)+ci, free = (slot=row//4+1, col+1), zero halo
    baked in (slots 0, 65, 66 and cols 0, 257 are zero).
  - conv as DoubleRow fp8 matmul PAIRS (cost-model rate 0.5 cy/row,
    two K=128 matmuls fused): per 4-row window, 3 pairs with N=256:
      P0 = (main dx0, main dx1)   rhs k-tiles 1 col apart
      P1 = (main dx2, wrap dx0)   rhs k-tiles WP-2 apart (next slot)
      P2 = (wrap dx1, wrap dx2)   rhs k-tiles 1 col apart
    The paired rhs view is one 3-dim AP [128, 2, 256] built by giving
    the k-tile dim a custom stride over the same x_il tile, so no
    second x copy (and no x_wrap HBM read) is needed.
  - weights pre-scaled by 64 into fp8 range; epilogue scale = g/64.
  - conv1 epilogue on ScalarE: h = relu(psum*g/64 + b1*g) -> fp8 h_il.
  - conv2 epilogue on VectorE in shifted max-form (1 instruction):
    h2' = max(psum*g/64, -b2*g) = relu(psum*g/64 + b2*g) - b2*g.
    The constant b2*g is restored host-side together with the residual:
    out = h2' + b2*g + x  (all f32 on host). The two tail chunks run on
    ScalarE in true-relu form instead (bias pre-applied; the host skips
    the +b2*g for their rows) so the drain never serializes on VectorE.
  - PSUM: one 4-buffer pool of [128, 4, 256] tiles (4 windows each)
    shared by both convs; conv1/conv2 pipelined with a 2-chunk lag and
    the two images software-pipelined so conv2's tail drains under the
    next image's conv1; epilogues batch 4 windows/instr.
  - out staged fp8, stored in 8-slot chunks (Pool engine) with finer
    tail stores spread over SP so no engine queue delays the drain.
"""

import numpy as np
import ml_dtypes

import concourse.bass as bass
import concourse.tile as tile
from concourse import bacc, mybir

B, C, H, W = 16, 32, 256, 256
IMGS_PER_CORE = 2
N_CORES = 8
KW = 3
S = 4            # row interleave factor (rows per window/slot)
A = H // S       # 64 aligned 4-row windows
WP = W + 2       # padded row width (zero cols 0 and 257)
NSX = A + 3      # x_il slots: idx = window + 1; idx 0, A+1, A+2 zero
NSR = A + 2      # out_stage slots
F32 = mybir.dt.float32
FP8 = mybir.dt.float8e4
E4 = ml_dtypes.float8_e4m3
WSCALE = 64.0    # weight pre-scale into fp8-normal range
DR = mybir.MatmulPerfMode.DoubleRow

# conv windows k = -1 .. 63, in chunks of (up to) 4; the tail is split
# finer so the post-conv1 drain (conv2 mm -> ep -> store) pipelines short
CHUNKS = ([[-1, 0, 1, 2]]
          + [list(range(s, s + 4)) for s in range(3, 56, 4)]
          + [[59, 60], [61, 62], [A - 1]])
# x_il slot ranges per DMA chunk; conv chunk c needs slots <= 4c+4, and
# boundaries are tuned so each chunk lands just before the PE consumes it
XCHUNKS = [(0, 3), (3, 7), (7, 15), (15, 25), (25, 41), (41, 57), (57, NSX)]


def _pack_weights(w: np.ndarray) -> np.ndarray:
    """w: [C_out, C_in, 3, 3] (OIHW) -> [6, 128, 128] lhsT stack.

    Block (s, q) of main[dx] = w[:, :, s-q, dx].T   (0 <= s-q <= 2)
    Block (s, q) of wrap[dx] = w[:, :, 4+s-q, dx].T (0 <= 4+s-q <= 2)
    lhsT[(32s+ci), (32q+co)]; out row (window k) = 4k+1+q.
    """
    wv = np.zeros((2 * KW, S * C, S * C), dtype=np.float32)
    for dx in range(KW):
        for q in range(S):
            for s in range(S):
                if 0 <= s - q <= 2:
                    wv[2 * dx, 32 * s:32 * s + 32, 32 * q:32 * q + 32] = \
                        w[:, :, s - q, dx].T
                if 0 <= 4 + s - q <= 2:
                    wv[2 * dx + 1, 32 * s:32 * s + 32, 32 * q:32 * q + 32] = \
                        w[:, :, 4 + s - q, dx].T
    return wv


# DoubleRow pair -> (full[] index of k-tile0, k-tile1)
PAIR_IDX = [(0, 2), (4, 1), (3, 5)]   # (m0,m1), (m2,w0), (w1,w2)


def _pack_pairs(w: np.ndarray) -> np.ndarray:
    """-> [128, 3, 2, 128] fp8 lhsT pair stack (K partition-first)."""
    full = _pack_weights(w) * WSCALE
    wvp = np.stack([np.stack([full[a], full[b]]) for a, b in PAIR_IDX])
    # [3, 2, K, M] -> [K, 3, 2, M]
    return np.ascontiguousarray(wvp.transpose(2, 0, 1, 3)).astype(E4)


def _interleave_x(x: np.ndarray) -> np.ndarray:
    """x: [n, C, H, W] f32 -> x_il [n, 128, NSX, WP] fp8.

    x_il: partition 32s+ci holds row 4(i-1)+s at slot i, col c+1.
    """
    n = x.shape[0]
    xq = x.astype(E4)
    ext = np.zeros((n, C, S * NSX, W), dtype=E4)
    ext[:, :, S:S + H, :] = xq
    il = ext.reshape(n, C, NSX, S, W).transpose(0, 3, 1, 2, 4) \
            .reshape(n, S * C, NSX, W)
    x_il = np.zeros((n, S * C, NSX, WP), dtype=E4)
    x_il[:, :, :, 1:1 + W] = il
    return np.ascontiguousarray(x_il)


def _deinterleave_out(dev: np.ndarray) -> np.ndarray:
    """dev: [n, 128, NSR, W] (row z = 4(i-1)+2+q at partition 32q+co)
    -> [n, C, H, W] f32."""
    dev = np.asarray(dev).astype(np.float32)
    n = dev.shape[0]
    v = dev.reshape(n, S, C, NSR, W).transpose(0, 2, 3, 1, 4) \
           .reshape(n, C, S * NSR, W)
    return np.ascontiguousarray(v[:, :, 2:2 + H, :])


def _build_core_graph():
    nc = bacc.Bacc(None, target_bir_lowering=False, debug=False)

    xil_ext = nc.declare_dram_parameter(
        "xil", [IMGS_PER_CORE, S * C, NSX, WP], FP8, isOutput=False)
    wv1_ext = nc.declare_dram_parameter(
        "wv1", [S * C, KW, 2, S * C], FP8, isOutput=False)
    wv2_ext = nc.declare_dram_parameter(
        "wv2", [S * C, KW, 2, S * C], FP8, isOutput=False)
    # sc cols: 0-1 g/WSCALE, 2-3 b1*g, 4-5 -(b2*g)   (per image)
    sc_ext = nc.declare_dram_parameter("sc", [S * C, 8], F32, isOutput=False)
    out_ext = nc.declare_dram_parameter(
        "out", [IMGS_PER_CORE, S * C, NSR, W], FP8, isOutput=True)

    RELU = mybir.ActivationFunctionType.Relu
    MULT = mybir.AluOpType.mult
    MAX = mybir.AluOpType.max

    with tile.TileContext(nc) as tc:
        with (
            tc.tile_pool(name="const", bufs=1) as cpool,
            tc.tile_pool(name="xb", bufs=2) as xpool,
            tc.tile_pool(name="hb", bufs=1) as hpool,
            tc.tile_pool(name="os", bufs=2) as ospool,
            tc.tile_pool(name="pp", bufs=4, space=bass.MemorySpace.PSUM) as psp,
        ):
            wv1_t = cpool.tile([S * C, KW, 2, S * C], FP8)
            wv2_t = cpool.tile([S * C, KW, 2, S * C], FP8)
            sc_t = cpool.tile([S * C, 8], F32)
            h_il = hpool.tile([S * C, NSX, WP], FP8)

            # consts spread over Pool/ACT so SP's serial queue is pure x
            # streaming (first matmul needs wv1 + x slots 0:3 ASAP); wv2 on
            # ACT behind its act-table load, still ready before conv2
            nc.gpsimd.dma_start(out=wv1_t[:], in_=wv1_ext[:])
            nc.gpsimd.dma_start(out=sc_t[:], in_=sc_ext[:])
            nc.scalar.dma_start(out=wv2_t[:], in_=wv2_ext[:])

            # h halo: never written by epilogues, init once (Pool engine).
            # slot 0 (window -1 writes only partitions 96:128 later), the
            # q=3 strip of slot 64 (row 256 pad), slots 65/66, cols 0/257.
            nc.gpsimd.memset(h_il[:, 0, :], 0.0)
            nc.gpsimd.memset(h_il[3 * C:4 * C, A, :], 0.0)
            nc.gpsimd.memset(h_il[:, A + 1, :], 0.0)
            nc.gpsimd.memset(h_il[:, A + 2, :], 0.0)
            nc.gpsimd.memset(h_il[:, :, 0], 0.0)
            nc.gpsimd.memset(h_il[:, :, WP - 1], 0.0)

            def pair_rhs(src, sl, col, delta):
                base = src[:, sl, col:col + W]
                v = base.unsqueeze(1).broadcast_to((S * C, 2, W))
                av = v.ap
                av[1] = [delta, 2]
                v.ap = av
                return v

            # pair p -> (base slot offset from k, base col, k-tile delta)
            PAIR_GEO = [(1, 0, 1), (1, 2, WP - 2), (2, 1, 1)]

            def conv_chunk(src, wv_t, ws, ps, conv_idx):
                for j, k in enumerate(ws):
                    if k == -1:
                        plist = [1, 2] if conv_idx == 1 else [0, 1, 2]
                    elif k == A - 1:
                        plist = [0, 1]
                    else:
                        plist = [0, 1, 2]
                    for n, p in enumerate(plist):
                        soff, col, delta = PAIR_GEO[p]
                        nc.tensor.matmul(
                            ps[:, j, :], wv_t[:, p, :, :],
                            pair_rhs(src, k + soff, col, delta),
                            start=(n == 0), stop=(n == len(plist) - 1),
                            perf_mode=DR, skip_group_check=True)

            def ep1(ps, ws, img):
                # h = relu(psum*g/64 + b1*g), true form, fp8 into h_il.
                # One full-range instruction per chunk (split partition
                # ranges invite scheduler reordering that stalls the PE);
                # the halo strip clobbered by window -1 is re-zeroed after.
                # The single-window tail chunk writes partitions 0:96 only
                # so the slot-64 halo strip stays pristine (a re-zero there
                # would gate the last conv2 matmuls)
                s0 = ws[0] + 1
                p1 = 3 * C if ws[-1] == A - 1 else 4 * C
                nc.scalar.activation(
                    h_il[0:p1, s0:s0 + len(ws), 1:1 + W],
                    ps[0:p1, 0:len(ws), :], RELU,
                    bias=sc_t[0:p1, 2 + img:3 + img],
                    scale=sc_t[0:p1, img:img + 1])
                if ws[0] == -1:
                    nc.gpsimd.memset(h_il[0:3 * C, 0, :], 0.0)

            def ep2(ps, ws, img, out_stage, eng):
                # h2' = max(psum*g/64, -b2*g); host adds back b2*g (+x).
                # eng=scalar: true-relu form on ScalarE (tail only); host
                # skips +b2*g for its rows (254-255)
                # full 128-partition writes: the discarded edge rows get
                # garbage, which the host slices off anyway
                s0 = ws[0] + 1
                nj = len(ws)
                if eng is nc.scalar:
                    nc.scalar.activation(
                        out_stage[:, s0:s0 + nj, :],
                        ps[:, 0:nj, :], RELU,
                        bias=sc_t[:, 6 + img:7 + img],
                        scale=sc_t[:, img:img + 1])
                    return
                negb = sc_t[:, 4 + img:5 + img] \
                    .unsqueeze(2).broadcast_to((S * C, nj, W))
                eng.scalar_tensor_tensor(
                    out_stage[:, s0:s0 + nj, :],
                    ps[:, 0:nj, :],
                    sc_t[:, img:img + 1],
                    negb, MULT, MAX)

            NCH = len(CHUNKS)
            stage = {}  # img -> (x_il, out_stage, last_stored_slot)

            def setup_img(img):
                x_il = xpool.tile([S * C, NSX, WP], FP8)
                out_stage = ospool.tile([S * C, NSR, W], FP8)
                for c0, c1 in XCHUNKS:
                    nc.sync.dma_start(out=x_il[:, c0:c1, :],
                                      in_=xil_ext[img, :, c0:c1, :])
                stage[img] = [x_il, out_stage, 0]

            def conv1_chunk(img, ci):
                ws = CHUNKS[ci]
                ps = psp.tile([S * C, S, W], F32, tag="ps")
                conv_chunk(stage[img][0], wv1_t, ws, ps, 1)
                ep1(ps, ws, img)

            def conv2_chunk(img, ci, tail_act=False):
                ws = CHUNKS[ci]
                out_stage = stage[img][1]
                ps = psp.tile([S * C, S, W], F32, tag="ps")
                conv_chunk(h_il, wv2_t, ws, ps, 2)
                # GPSIMD cannot touch PSUM (BIR verifier), so epilogues live
                # on DVE with the c15/c17 tail chunks on ScalarE (true-relu
                # form) so the drain doesn't serialize through DVE; for the
                # last image c16 also runs on ScalarE (DVE is still chewing
                # on c14 when the final store needs it)
                if ci == NCH - 1 or ci == NCH - 3 or \
                        (tail_act and ci == NCH - 2):
                    eng = nc.scalar
                else:
                    eng = nc.vector
                ep2(ps, ws, img, out_stage, eng)
                # store completed slot ranges (out slot = window+1): 8-slot
                # chunks mid-stream from Pool; the tail stores spread over
                # SP/Pool/ACT so no engine queue delays the drain
                if ci == NCH - 1:
                    st = nc.scalar if tail_act else nc.sync
                    st.dma_start(
                        out=out_ext[img, :, A:A + 1, :],
                        in_=out_stage[:, A:A + 1, :])
                    return
                hi = ws[-1] + 2
                lo = stage[img][2]
                if ci == NCH - 5 or ci == NCH - 2:
                    st_eng = nc.sync
                elif ci == NCH - 4 or hi - lo >= 8:
                    st_eng = nc.gpsimd
                else:
                    st_eng = None
                if st_eng is not None:
                    st_eng.dma_start(
                        out=out_ext[img, :, lo:hi, :],
                        in_=out_stage[:, lo:hi, :])
                    stage[img][2] = hi

            # flat software pipeline over both images: conv2 lags conv1 by
            # 2 chunks (its h slots come from conv1 chunk ci+1's epilogue),
            # and each image's conv2 tail drains under the next image's
            # conv1 head so the PE never idles at the boundary
            setup_img(0)
            for img in range(IMGS_PER_CORE):
                if img + 1 < IMGS_PER_CORE:
                    setup_img(img + 1)
                for ci in range(NCH):
                    conv1_chunk(img, ci)
                    if ci >= 2:
                        conv2_chunk(img, ci - 2)
                    elif img > 0:
                        conv2_chunk(img - 1, NCH - 2 + ci)
            conv2_chunk(IMGS_PER_CORE - 1, NCH - 2, tail_act=True)
            conv2_chunk(IMGS_PER_CORE - 1, NCH - 1, tail_act=True)

    nc.compile()
    return nc


def _host_prep(x, gate_values, w1, b1, w2, b2):
    x = np.ascontiguousarray(np.asarray(x, dtype=np.float32))
    gate_values = np.asarray(gate_values, dtype=np.float32)
    w1 = np.asarray(w1, dtype=np.float32)
    b1 = np.asarray(b1, dtype=np.float32)
    w2 = np.asarray(w2, dtype=np.float32)
    b2 = np.asarray(b2, dtype=np.float32)

    g = gate_values * (gate_values > 0)                      # [B, C]
    wv1 = _pack_pairs(w1)
    wv2 = _pack_pairs(w2)

    in_maps = []
    for core in range(N_CORES):
        sl = slice(core * IMGS_PER_CORE, (core + 1) * IMGS_PER_CORE)
        gc = g[sl]                                           # [2, C]
        sc = np.zeros((S * C, 8), dtype=np.float32)
        sc[:, 0:2] = np.tile((gc / WSCALE).T, (S, 1))
        sc[:, 2:4] = np.tile((gc * b1[None, :]).T, (S, 1))
        sc[:, 4:6] = np.tile((-gc * b2[None, :]).T, (S, 1))
        sc[:, 6:8] = np.tile((gc * b2[None, :]).T, (S, 1))
        in_maps.append({
            "xil": _interleave_x(x[sl]),
            "wv1": wv1, "wv2": wv2,
            "sc": np.ascontiguousarray(sc),
        })
    return in_maps


_NC_CACHE = None


def _get_graph():
    global _NC_CACHE
    if _NC_CACHE is None:
        _NC_CACHE = _build_core_graph()
    return _NC_CACHE


def kernel(x, gate_values, w1, b1, w2, b2, _trace=False, **_ignored):
    from concourse.bass_utils import run_bass_kernel_spmd

    nc = _get_graph()
    in_maps = _host_prep(x, gate_values, w1, b1, w2, b2)
    res = run_bass_kernel_spmd(
        nc, in_maps, core_ids=list(range(N_CORES)), trace=_trace)
    outs = [_deinterleave_out(res.results[i]["out"]) for i in range(N_CORES)]
    full = np.concatenate(outs, axis=0).astype(np.float32)
    # restore the shifted conv2 bias and add the residual (f32, host-side).
    # rows whose tail epilogue ran on ScalarE in true-relu form already
    # have the bias applied, so they are excluded here
    g = np.asarray(gate_values, dtype=np.float32)
    g = g * (g > 0)
    bg2 = (g * np.asarray(b2, dtype=np.float32)[None, :])[:, :, None, None]
    mask = np.ones(H, dtype=bool)
    for w in CHUNKS[-3] + CHUNKS[-1]:
        mask[4 * w + 2:4 * w + 6] = False
    # each core's second image also ran the c16 epilogue in true-relu form
    mask_odd = mask.copy()
    for w in CHUNKS[-2]:
        mask_odd[4 * w + 2:4 * w + 6] = False
    full[0::2][:, :, mask, :] += bg2[0::2]
    full[1::2][:, :, mask_odd, :] += bg2[1::2]
    full += np.asarray(x, dtype=np.float32)
    if _trace:
        return full, res
    return full
